# revision 2
# baseline (speedup 1.0000x reference)
"""GreenTF filterbank (strided sinusoid conv) on 8 trn2 NeuronCores.

reference:  k = kernel*envelope/SR;  frames = im2col(pad(wav), K=2048, stride=16)
            spec = einsum('btk,fk->bft', frames, k) * sqrt(8)/(sum(envelope)/SR)
            returns (spec[:, :8001], spec[:, 8001:])   # each [2, 8001, 2000]

Strategy: shard the 8001 frequency bands across the 8 cores (sin+cos rows of
the same band live together).  Host does layout only: transpose the kernel
slice to [K, f] (contraction on partitions), im2col+transpose wav to
framesT [K, 4000], dtype pre-round.  Device applies envelope*scale to the
kernel slice and runs the tiled GEMM out[f, t] = kT.T @ framesT with fp32
PSUM accumulation.
"""

import math
import os

os.environ.setdefault("MYCRO_LOCAL_CACHE", "1")

import numpy as np

import concourse.bass as bass  # noqa: F401  (engine handles live on the Bacc object)
import concourse.mybir as mybir
from concourse import bacc
from concourse.bass_utils import run_bass_kernel_spmd
from concourse.tile import TileContext

SR = 16000
KSIZE = 2048
F = 8001          # frequencies; rows 0:F sin, F:2F cos
B = 2
T_OUT = 2000
STRIDE = 16
N_CORES = 8
NT = B * T_OUT    # 4000 columns, batch-major
CHUNK = 400       # t-columns per matmul (PSUM free dim), 10 chunks
N_CHUNKS = NT // CHUNK
KT_TILES = KSIZE // 128   # 16 contraction tiles
FT_TILES = 16             # 2048 padded f-rows per core
FPAD = FT_TILES * 128

# f-band sizes per core: 8001 = 1001 + 7*1000
_BAND = [1001] + [1000] * 7
_OFF = np.cumsum([0] + _BAND)

VARIANT = os.environ.get("GREENTF_VARIANT", "f16")  # "f16" | "f32r"
TRACE = bool(int(os.environ.get("GREENTF_TRACE", "0")))

_prog_cache = {}


def _build_program(variant):
    nc = bacc.Bacc()
    fp32 = mybir.dt.float32
    cdt = mybir.dt.float16 if variant == "f16" else mybir.dt.float32r

    kt_d = nc.dram_tensor("kt", [KSIZE, FPAD], fp32, kind="ExternalInput").ap()
    fr_d = nc.dram_tensor("fr", [KSIZE, NT], cdt, kind="ExternalInput").ap()
    env_d = nc.dram_tensor("env", [128, KT_TILES], fp32, kind="ExternalInput").ap()
    out_d = nc.dram_tensor("out", [FPAD, NT], fp32, kind="ExternalOutput").ap()

    post = (1.0 / 256.0) if variant == "f16" else 1.0

    with TileContext(nc) as tc:
        with (
            tc.tile_pool(name="ktres", bufs=1) as ktres_p,
            tc.tile_pool(name="ktstage", bufs=2) as ktstage_p,
            tc.tile_pool(name="env", bufs=1) as env_p,
            tc.tile_pool(name="fr", bufs=2) as fr_p,
            tc.tile_pool(name="ostage", bufs=4) as ostage_p,
            tc.tile_pool(name="ps", bufs=4, space="PSUM") as ps_p,
        ):
            env_t = env_p.tile([128, KT_TILES], fp32)
            nc.sync.dma_start(out=env_t[:], in_=env_d)

            ktres = ktres_p.tile([128, KT_TILES, FPAD], cdt)
            for kt in range(KT_TILES):
                dst = ktres[:, kt, :]
                if variant == "f16":
                    raw = ktstage_p.tile([128, FPAD], fp32)
                    nc.sync.dma_start(out=raw[:], in_=kt_d[kt * 128:(kt + 1) * 128, :])
                    src = raw[:]
                else:
                    # stream raw fp32 into the f32r tile, then round in place
                    nc.sync.dma_start(
                        out=dst.bitcast(fp32), in_=kt_d[kt * 128:(kt + 1) * 128, :]
                    )
                    src = dst.bitcast(fp32)
                nc.vector.tensor_scalar_mul(dst, src, env_t[:, kt:kt + 1])

            for c in range(N_CHUNKS):
                frt = fr_p.tile([128, KT_TILES, CHUNK], cdt)
                cs = slice(c * CHUNK, (c + 1) * CHUNK)
                for kt in range(KT_TILES):
                    nc.sync.dma_start(
                        out=frt[:, kt, :], in_=fr_d[kt * 128:(kt + 1) * 128, cs]
                    )
                for ft in range(FT_TILES):
                    pt = ps_p.tile([128, CHUNK], fp32)
                    for kt in range(KT_TILES):
                        nc.tensor.matmul(
                            pt[:],
                            ktres[:, kt, ft * 128:(ft + 1) * 128],
                            frt[:, kt, :],
                            start=(kt == 0),
                            stop=(kt == KT_TILES - 1),
                        )
                    ot = ostage_p.tile([128, CHUNK], fp32)
                    nc.any.tensor_scalar_mul(ot[:], pt[:], post)
                    nc.sync.dma_start(
                        out=out_d[ft * 128:(ft + 1) * 128, cs], in_=ot[:]
                    )

    nc.finalize()
    return nc


def _round_f32r(x: np.ndarray) -> np.ndarray:
    u = x.astype(np.float32).view(np.uint32).copy()
    lsb = (u >> 12) & 1
    u += 0x7FF + lsb
    u &= 0xFFFFF000
    return u.view(np.float32)


def kernel(wav: np.ndarray, kernel: np.ndarray, envelope: np.ndarray):
    assert wav.shape == (B, T_OUT * STRIDE) and kernel.shape == (2 * F, KSIZE)
    variant = VARIANT

    # --- host layout prep (sharding + transposes only; arithmetic on device) ---
    # scaled envelope: envelope/SR * sqrt(8)/envelope_area (+2^8 for f16 range)
    env = envelope.astype(np.float64)
    scale = math.sqrt(8.0) / (env.sum() / SR)
    env_dev = (env / SR * scale * (256.0 if variant == "f16" else 1.0)).astype(
        np.float32
    )
    env2d = np.ascontiguousarray(env_dev.reshape(KT_TILES, 128).T)  # [128, kt]

    # im2col: framesT[i, b*2000+t] = pad[b, 16t+i]
    pad = np.zeros((B, KSIZE - 1 + wav.shape[1]), np.float32)
    pad[:, KSIZE - 1:] = wav
    frames = np.lib.stride_tricks.sliding_window_view(pad, KSIZE, axis=1)[:, ::STRIDE]
    fr = np.ascontiguousarray(frames.transpose(2, 0, 1).reshape(KSIZE, NT))
    fr = fr.astype(np.float16) if variant == "f16" else _round_f32r(fr)

    in_maps = []
    for c in range(N_CORES):
        f0, fc = int(_OFF[c]), _BAND[c]
        kt = np.zeros((KSIZE, FPAD), np.float32)
        kt[:, :fc] = kernel[f0:f0 + fc].T
        kt[:, fc:2 * fc] = kernel[F + f0:F + f0 + fc].T
        in_maps.append({"kt": kt, "fr": fr, "env": env2d})

    key = variant
    if key not in _prog_cache:
        _prog_cache[key] = _build_program(variant)
    nc = _prog_cache[key]

    kwargs = {}
    if TRACE:
        kwargs["tmpdir"] = os.environ.get("GREENTF_TRACE_DIR") or None
    res = run_bass_kernel_spmd(nc, in_maps, list(range(N_CORES)), trace=TRACE, **kwargs)
    if TRACE:
        print(f"HW exec time: {res.exec_time_ns} ns "
              f"(mean {res.mean_exec_time_ns} ns, core {res.max_exec_time_core_id})")

    sspec = np.empty((B, F, T_OUT), np.float32)
    cspec = np.empty((B, F, T_OUT), np.float32)
    for c in range(N_CORES):
        f0, fc = int(_OFF[c]), _BAND[c]
        o = res.results[c]["out"]
        for b in range(B):
            cols = slice(b * T_OUT, (b + 1) * T_OUT)
            sspec[b, f0:f0 + fc] = o[:fc, cols]
            cspec[b, f0:f0 + fc] = o[fc:2 * fc, cols]
    return sspec, cspec


# revision 7
# speedup vs baseline: 1.0968x; 1.0968x over previous
"""GreenTF filterbank (strided sinusoid conv) on 8 trn2 NeuronCores.

reference:  k = kernel*envelope/SR;  frames = im2col(pad(wav), K=2048, stride=16)
            spec = einsum('btk,fk->bft', frames, k) * sqrt(8)/(sum(envelope)/SR)
            returns (spec[:, :8001], spec[:, 8001:])   # each [2, 8001, 2000]

Strategy: shard the 8001 frequency bands across the 8 cores (sin+cos rows of
the same band live together).  Host does layout only: transpose the kernel
slice to [K, f] (contraction on partitions), im2col+transpose wav to
framesT [K, 4000], dtype pre-round.  Device applies envelope*scale to the
kernel slice and runs the tiled GEMM out[f, t] = kT.T @ framesT with fp32
PSUM accumulation.
"""

import math
import os

os.environ.setdefault("MYCRO_LOCAL_CACHE", "1")

import numpy as np

import concourse.bass as bass  # noqa: F401  (engine handles live on the Bacc object)
import concourse.mybir as mybir
from concourse import bacc
from concourse.bass_utils import run_bass_kernel_spmd
from concourse.tile import TileContext

SR = 16000
KSIZE = 2048
F = 8001          # frequencies; rows 0:F sin, F:2F cos
B = 2
T_OUT = 2000
STRIDE = 16
N_CORES = 8
NT = B * T_OUT    # 4000 columns, batch-major
CHUNK = 400       # t-columns per matmul (PSUM free dim), 10 chunks
N_CHUNKS = NT // CHUNK
KT_TILES = KSIZE // 128   # 16 contraction tiles
FT_TILES = 16             # 2048 padded f-rows per core
FPAD = FT_TILES * 128

# f-band sizes per core: 8001 = 1001 + 7*1000
_BAND = [1001] + [1000] * 7
_OFF = np.cumsum([0] + _BAND)

VARIANT = os.environ.get("GREENTF_VARIANT", "f16")  # "f16" | "f32r"
TRACE = bool(int(os.environ.get("GREENTF_TRACE", "0")))

_prog_cache = {}


def _build_program(variant):
    nc = bacc.Bacc()
    fp32 = mybir.dt.float32
    f16 = variant == "f16"
    cdt = mybir.dt.float16 if f16 else mybir.dt.float32r

    kt_d = nc.dram_tensor("kt", [KSIZE, FPAD],
                          mybir.dt.float16 if f16 else fp32,
                          kind="ExternalInput").ap()
    fr_d = nc.dram_tensor("fr", [KSIZE, NT], cdt, kind="ExternalInput").ap()
    env_d = nc.dram_tensor("env", [128, KT_TILES], fp32, kind="ExternalInput").ap()
    out_d = nc.dram_tensor("out", [FPAD, NT], fp32, kind="ExternalOutput").ap()

    post = (1.0 / 256.0) if f16 else 1.0

    with TileContext(nc) as tc:
        with (
            tc.tile_pool(name="ktres", bufs=1) as ktres_p,
            tc.tile_pool(name="env", bufs=1) as env_p,
            tc.tile_pool(name="fr", bufs=2) as fr_p,
            tc.tile_pool(name="ostage", bufs=4) as ostage_p,
            tc.tile_pool(name="ps", bufs=8, space="PSUM") as ps_p,
        ):
            env_t = env_p.tile([128, KT_TILES], fp32)
            nc.sync.dma_start(out=env_t[:], in_=env_d)

            # kernel-slice prep, interleaved with chunk-0 frames so the PE
            # can start as soon as k-tile 0 lands
            frt0 = fr_p.tile([128, KT_TILES, CHUNK], cdt, tag="fr")
            ktres = []
            for kt in range(KT_TILES):
                t = ktres_p.tile([128, FPAD], cdt, tag=f"kt{kt}")
                if f16:
                    nc.sync.dma_start(out=t[:], in_=kt_d[kt * 128:(kt + 1) * 128, :])
                    src = t[:]
                else:
                    nc.sync.dma_start(
                        out=t[:].bitcast(fp32), in_=kt_d[kt * 128:(kt + 1) * 128, :]
                    )
                    src = t[:].bitcast(fp32)
                nc.sync.dma_start(
                    out=frt0[:, kt, :], in_=fr_d[kt * 128:(kt + 1) * 128, 0:CHUNK]
                )
                nc.vector.tensor_scalar_mul(t[:], src, env_t[:, kt:kt + 1])
                ktres.append(t)

            def mm(pt, ft, kt, frt):
                nc.tensor.matmul(
                    pt[:],
                    ktres[kt][:, ft * 128:(ft + 1) * 128],
                    frt[:, kt, :],
                    start=(kt == 0),
                    stop=(kt == KT_TILES - 1),
                )

            def evict(pt, ft, c):
                ot = ostage_p.tile([128, CHUNK], fp32, tag="ostage")
                nc.any.tensor_scalar_mul(ot[:], pt[:], post)
                nc.sync.dma_start(
                    out=out_d[ft * 128:(ft + 1) * 128,
                              c * CHUNK:(c + 1) * CHUNK],
                    in_=ot[:],
                )

            # chunk 0: kt-outer over 8 PSUM banks per half, so matmuls
            # consume kernel tiles at DMA arrival rate
            for g in range(2):
                fts = list(range(g * 8, g * 8 + 8))
                pts = {}
                for ft in fts:
                    pts[ft] = ps_p.tile([128, CHUNK], fp32, tag="ps", name=f"pt{ft}")
                for kt in range(KT_TILES):
                    for ft in fts:
                        mm(pts[ft], ft, kt, frt0)
                for ft in fts:
                    evict(pts[ft], ft, 0)

            # steady-state chunks
            for c in range(1, N_CHUNKS):
                frt = fr_p.tile([128, KT_TILES, CHUNK], cdt, tag="fr")
                cs = slice(c * CHUNK, (c + 1) * CHUNK)
                for kt in range(KT_TILES):
                    nc.sync.dma_start(
                        out=frt[:, kt, :], in_=fr_d[kt * 128:(kt + 1) * 128, cs]
                    )
                for ft in range(FT_TILES):
                    pt = ps_p.tile([128, CHUNK], fp32, tag="ps")
                    for kt in range(KT_TILES):
                        mm(pt, ft, kt, frt)
                    evict(pt, ft, c)

    nc.finalize()
    return nc


def _round_f32r(x: np.ndarray) -> np.ndarray:
    u = x.astype(np.float32).view(np.uint32).copy()
    lsb = (u >> 12) & 1
    u += 0x7FF + lsb
    u &= 0xFFFFF000
    return u.view(np.float32)


def kernel(wav: np.ndarray, kernel: np.ndarray, envelope: np.ndarray):
    assert wav.shape == (B, T_OUT * STRIDE) and kernel.shape == (2 * F, KSIZE)
    variant = VARIANT

    # --- host layout prep (sharding + transposes only; arithmetic on device) ---
    # scaled envelope: envelope/SR * sqrt(8)/envelope_area (+2^8 for f16 range)
    env = envelope.astype(np.float64)
    scale = math.sqrt(8.0) / (env.sum() / SR)
    env_dev = (env / SR * scale * (256.0 if variant == "f16" else 1.0)).astype(
        np.float32
    )
    env2d = np.ascontiguousarray(env_dev.reshape(KT_TILES, 128).T)  # [128, kt]

    # im2col: framesT[i, b*2000+t] = pad[b, 16t+i]
    pad = np.zeros((B, KSIZE - 1 + wav.shape[1]), np.float32)
    pad[:, KSIZE - 1:] = wav
    frames = np.lib.stride_tricks.sliding_window_view(pad, KSIZE, axis=1)[:, ::STRIDE]
    fr = np.ascontiguousarray(frames.transpose(2, 0, 1).reshape(KSIZE, NT))
    fr = fr.astype(np.float16) if variant == "f16" else _round_f32r(fr)

    in_maps = []
    for c in range(N_CORES):
        f0, fc = int(_OFF[c]), _BAND[c]
        kt = np.zeros((KSIZE, FPAD), np.float32)
        kt[:, :fc] = kernel[f0:f0 + fc].T
        kt[:, fc:2 * fc] = kernel[F + f0:F + f0 + fc].T
        if variant == "f16":
            kt = kt.astype(np.float16)
        in_maps.append({"kt": kt, "fr": fr, "env": env2d})

    key = variant
    if key not in _prog_cache:
        _prog_cache[key] = _build_program(variant)
    nc = _prog_cache[key]

    kwargs = {}
    if TRACE:
        kwargs["tmpdir"] = os.environ.get("GREENTF_TRACE_DIR") or None
    res = run_bass_kernel_spmd(nc, in_maps, list(range(N_CORES)), trace=TRACE, **kwargs)
    if TRACE:
        print(f"HW exec time: {res.exec_time_ns} ns "
              f"(mean {res.mean_exec_time_ns} ns, core {res.max_exec_time_core_id})")

    sspec = np.empty((B, F, T_OUT), np.float32)
    cspec = np.empty((B, F, T_OUT), np.float32)
    for c in range(N_CORES):
        f0, fc = int(_OFF[c]), _BAND[c]
        o = res.results[c]["out"]
        for b in range(B):
            cols = slice(b * T_OUT, (b + 1) * T_OUT)
            sspec[b, f0:f0 + fc] = o[:fc, cols]
            cspec[b, f0:f0 + fc] = o[fc:2 * fc, cols]
    return sspec, cspec


# revision 9
# speedup vs baseline: 1.1080x; 1.0101x over previous
"""GreenTF filterbank (strided sinusoid conv) on 8 trn2 NeuronCores.

reference:  k = kernel*envelope/SR;  frames = im2col(pad(wav), K=2048, stride=16)
            spec = einsum('btk,fk->bft', frames, k) * sqrt(8)/(sum(envelope)/SR)
            returns (spec[:, :8001], spec[:, 8001:])   # each [2, 8001, 2000]

Strategy: shard the 8001 frequency bands across the 8 cores (sin+cos rows of
the same band live together).  Host does layout only: transpose the kernel
slice to [K, f] (contraction on partitions), im2col+transpose wav to
framesT [K, 4000], dtype pre-round.  Device applies envelope*scale to the
kernel slice and runs the tiled GEMM out[f, t] = kT.T @ framesT with fp32
PSUM accumulation.
"""

import math
import os

os.environ.setdefault("MYCRO_LOCAL_CACHE", "1")

import numpy as np

import concourse.bass as bass  # noqa: F401  (engine handles live on the Bacc object)
import concourse.mybir as mybir
from concourse import bacc
from concourse.bass_utils import run_bass_kernel_spmd
from concourse.tile import TileContext

SR = 16000
KSIZE = 2048
F = 8001          # frequencies; rows 0:F sin, F:2F cos
B = 2
T_OUT = 2000
STRIDE = 16
N_CORES = 8
NT = B * T_OUT    # 4000 columns, batch-major
CHUNK = 500       # t-columns per matmul (PSUM free dim), 8 chunks
N_CHUNKS = NT // CHUNK
KT_TILES = KSIZE // 128   # 16 contraction tiles
FT_TILES = 16             # 2048 padded f-rows per core
FPAD = FT_TILES * 128

# f-band sizes per core: 8001 = 1001 + 7*1000
_BAND = [1001] + [1000] * 7
_OFF = np.cumsum([0] + _BAND)

VARIANT = os.environ.get("GREENTF_VARIANT", "f16")  # "f16" | "f32r"
TRACE = bool(int(os.environ.get("GREENTF_TRACE", "0")))

_prog_cache = {}


def _build_program(variant):
    nc = bacc.Bacc()
    fp32 = mybir.dt.float32
    f16 = variant == "f16"
    cdt = mybir.dt.float16 if f16 else mybir.dt.float32r

    kt_d = nc.dram_tensor("kt", [KSIZE, FPAD],
                          mybir.dt.float16 if f16 else fp32,
                          kind="ExternalInput").ap()
    fr_d = nc.dram_tensor("fr", [KSIZE, NT], cdt, kind="ExternalInput").ap()
    env_d = nc.dram_tensor("env", [128, KT_TILES], fp32, kind="ExternalInput").ap()
    out_d = nc.dram_tensor("out", [FPAD, NT], fp32, kind="ExternalOutput").ap()

    post = (1.0 / 256.0) if f16 else 1.0

    with TileContext(nc) as tc:
        with (
            tc.tile_pool(name="ktres", bufs=1) as ktres_p,
            tc.tile_pool(name="env", bufs=1) as env_p,
            tc.tile_pool(name="fr", bufs=2) as fr_p,
            tc.tile_pool(name="ostage", bufs=4) as ostage_p,
            tc.tile_pool(name="ps", bufs=8, space="PSUM") as ps_p,
        ):
            env_t = env_p.tile([128, KT_TILES], fp32)
            nc.sync.dma_start(out=env_t[:], in_=env_d)

            # PE warmup during the initial DMA wait: junk matmuls on a
            # memset scratch tile warm the HAM clock gate to 2.4 GHz
            warm = env_p.tile([128, 512], cdt, name="warm")
            nc.vector.memset(warm[:], 0.0)
            wps = ps_p.tile([128, 512], fp32, name="wps", tag="ps")
            for w in range(24):
                nc.tensor.matmul(wps[:], warm[:, 0:128], warm[:],
                                 start=(w == 0), stop=(w == 23))

            # kernel-slice prep, interleaved with chunk-0 frames so the PE
            # can start as soon as k-tile 0 lands
            frt0 = fr_p.tile([128, KT_TILES, CHUNK], cdt, tag="fr")
            ktres = []
            for kt in range(KT_TILES):
                t = ktres_p.tile([128, FPAD], cdt, tag=f"kt{kt}")
                if f16:
                    nc.sync.dma_start(out=t[:], in_=kt_d[kt * 128:(kt + 1) * 128, :])
                    src = t[:]
                else:
                    nc.sync.dma_start(
                        out=t[:].bitcast(fp32), in_=kt_d[kt * 128:(kt + 1) * 128, :]
                    )
                    src = t[:].bitcast(fp32)
                nc.sync.dma_start(
                    out=frt0[:, kt, :], in_=fr_d[kt * 128:(kt + 1) * 128, 0:CHUNK]
                )
                nc.vector.tensor_scalar_mul(t[:], src, env_t[:, kt:kt + 1])
                ktres.append(t)

            def mm(pt, ft, kt, frt):
                nc.tensor.matmul(
                    pt[:],
                    ktres[kt][:, ft * 128:(ft + 1) * 128],
                    frt[:, kt, :],
                    start=(kt == 0),
                    stop=(kt == KT_TILES - 1),
                )

            def evict(pt, ft, c):
                ot = ostage_p.tile([128, CHUNK], fp32, tag="ostage")
                nc.any.tensor_scalar_mul(ot[:], pt[:], post)
                nc.sync.dma_start(
                    out=out_d[ft * 128:(ft + 1) * 128,
                              c * CHUNK:(c + 1) * CHUNK],
                    in_=ot[:],
                )

            # chunk 0: kt-outer over 8 PSUM banks per half, so matmuls
            # consume kernel tiles at DMA arrival rate
            for g in range(2):
                fts = list(range(g * 8, g * 8 + 8))
                pts = {}
                for ft in fts:
                    pts[ft] = ps_p.tile([128, CHUNK], fp32, tag="ps", name=f"pt{ft}")
                for kt in range(KT_TILES):
                    for ft in fts:
                        mm(pts[ft], ft, kt, frt0)
                for ft in fts:
                    evict(pts[ft], ft, 0)

            # steady-state chunks
            for c in range(1, N_CHUNKS):
                frt = fr_p.tile([128, KT_TILES, CHUNK], cdt, tag="fr")
                cs = slice(c * CHUNK, (c + 1) * CHUNK)
                for kt in range(KT_TILES):
                    nc.sync.dma_start(
                        out=frt[:, kt, :], in_=fr_d[kt * 128:(kt + 1) * 128, cs]
                    )
                for ft in range(FT_TILES):
                    pt = ps_p.tile([128, CHUNK], fp32, tag="ps")
                    for kt in range(KT_TILES):
                        mm(pt, ft, kt, frt)
                    evict(pt, ft, c)

    nc.finalize()
    return nc


def _round_f32r(x: np.ndarray) -> np.ndarray:
    u = x.astype(np.float32).view(np.uint32).copy()
    lsb = (u >> 12) & 1
    u += 0x7FF + lsb
    u &= 0xFFFFF000
    return u.view(np.float32)


def kernel(wav: np.ndarray, kernel: np.ndarray, envelope: np.ndarray):
    assert wav.shape == (B, T_OUT * STRIDE) and kernel.shape == (2 * F, KSIZE)
    variant = VARIANT

    # --- host layout prep (sharding + transposes only; arithmetic on device) ---
    # scaled envelope: envelope/SR * sqrt(8)/envelope_area (+2^8 for f16 range)
    env = envelope.astype(np.float64)
    scale = math.sqrt(8.0) / (env.sum() / SR)
    env_dev = (env / SR * scale * (256.0 if variant == "f16" else 1.0)).astype(
        np.float32
    )
    env2d = np.ascontiguousarray(env_dev.reshape(KT_TILES, 128).T)  # [128, kt]

    # im2col: framesT[i, b*2000+t] = pad[b, 16t+i]
    pad = np.zeros((B, KSIZE - 1 + wav.shape[1]), np.float32)
    pad[:, KSIZE - 1:] = wav
    frames = np.lib.stride_tricks.sliding_window_view(pad, KSIZE, axis=1)[:, ::STRIDE]
    fr = np.ascontiguousarray(frames.transpose(2, 0, 1).reshape(KSIZE, NT))
    fr = fr.astype(np.float16) if variant == "f16" else _round_f32r(fr)

    in_maps = []
    for c in range(N_CORES):
        f0, fc = int(_OFF[c]), _BAND[c]
        kt = np.zeros((KSIZE, FPAD), np.float32)
        kt[:, :fc] = kernel[f0:f0 + fc].T
        kt[:, fc:2 * fc] = kernel[F + f0:F + f0 + fc].T
        if variant == "f16":
            kt = kt.astype(np.float16)
        in_maps.append({"kt": kt, "fr": fr, "env": env2d})

    key = variant
    if key not in _prog_cache:
        _prog_cache[key] = _build_program(variant)
    nc = _prog_cache[key]

    kwargs = {}
    if TRACE:
        kwargs["tmpdir"] = os.environ.get("GREENTF_TRACE_DIR") or None
    res = run_bass_kernel_spmd(nc, in_maps, list(range(N_CORES)), trace=TRACE, **kwargs)
    if TRACE:
        print(f"HW exec time: {res.exec_time_ns} ns "
              f"(mean {res.mean_exec_time_ns} ns, core {res.max_exec_time_core_id})")

    sspec = np.empty((B, F, T_OUT), np.float32)
    cspec = np.empty((B, F, T_OUT), np.float32)
    for c in range(N_CORES):
        f0, fc = int(_OFF[c]), _BAND[c]
        o = res.results[c]["out"]
        for b in range(B):
            cols = slice(b * T_OUT, (b + 1) * T_OUT)
            sspec[b, f0:f0 + fc] = o[:fc, cols]
            cspec[b, f0:f0 + fc] = o[fc:2 * fc, cols]
    return sspec, cspec
